# revision 2
# baseline (speedup 1.0000x reference)
"""Trainium2 Bass kernel for nn_BertContrastivePredictor.

Sharding: data-parallel over batch, 4 samples per core (8 cores).
Per core: 64 spans (fwd + bwd chains).

Structure (v2 — hoisted Wih, transposed gates):
  - Phase 1: gxT[dir, gt, :] = WihT-chunk @ xT for ALL 16 steps at once.
    Full 128x128 stationary (weight chunks), moving = xT token columns
    (N=512) -> stream-bound at the bf16 roofline (~109us model).
  - Phase 2: recurrence in gates-TRANSPOSED layout [gate-dim, span].
    Per step per dir: inject gxT slice via identity-stationary matmuls
    (start=True), then 64 Whh matmuls [128,128]x[128,64] accumulate.
    h stays transposed -> no per-step DMA transposes; c/h/sf updates on
    scalar+vector engines overlap the other direction's PE work.
  - Phase 3: attention identical to baseline (block-diagonal masked
    attn, pipelined by sample), but sfT now exists natively.

labels sim-part on host (precision; ~0.5% of FLOPs), as baseline.

Output per core [64, 2080] = [slot_feats(1024) | context(1024) | labels(32)].
"""

import contextlib

import numpy as np
import ml_dtypes

import concourse.bass as bass
import concourse.bacc as bacc
import concourse.tile as tile
import concourse.mybir as mybir
from concourse import bass_utils
from concourse import library_config

f32 = mybir.dt.float32
bf16 = mybir.dt.bfloat16
i16 = mybir.dt.int16
AF = mybir.ActivationFunctionType
OP = mybir.AluOpType

B, S, D, H, K, L, NS, NT = 32, 512, 1024, 512, 16, 16, 16, 16
SMOOTH = 0.1
EPS = 1e-8
NCORES = 8
BL = B // NCORES            # local batch = 4
NSP = BL * K                # local spans = 64
G4 = 4 * H                  # 2048 gates per direction
OUTW = 2 * H + D + NS + NT  # 2080
ROWS = BL * S               # 2048 hidden rows per core
PERM = (2, 0, 1, 3)         # torch gate order (i,f,g,o) -> (g,i,f,o)
NGT = 16                    # gate tiles of 128 per direction


def build_program(dbg=False, reps=1, has_bias=False):
    nc = bacc.Bacc("TRN2", target_bir_lowering=False, debug=False,
                   num_swdge_queues=4)

    hid_d = nc.dram_tensor("hid", [ROWS, D], bf16, kind="ExternalInput")
    gx_d = nc.dram_tensor("gx", [2, 128, 32], i16, kind="ExternalInput")
    gh_d = nc.dram_tensor("gh", [4, 128, 32], i16, kind="ExternalInput")
    wihT_d = nc.dram_tensor("wihT", [128, 2, NGT, 8, 128], bf16,
                            kind="ExternalInput")
    whhT_d = nc.dram_tensor("whhT", [128, 2, NGT, 4, 128], bf16,
                            kind="ExternalInput")
    cmask_d = nc.dram_tensor("cmask", [NSP, BL, S], bf16, kind="ExternalInput")
    labs_d = nc.dram_tensor("labs", [NSP, NS + NT], f32, kind="ExternalInput")
    idb_d = nc.dram_tensor("idb", [128, 128], bf16, kind="ExternalInput")
    biasT_d = nc.dram_tensor("biasT", [128, 2, NGT], f32, kind="ExternalInput")
    out_d = nc.dram_tensor("out", [NSP, OUTW], f32, kind="ExternalOutput")

    with tile.TileContext(nc, pool_alloc_mode="queue") as tc:
        with tc.tile_pool(name="cst", bufs=1) as cst:
            cs = {
                "cmask": cst.tile([NSP, BL * S], bf16, name="cmask"),
                "sfT": cst.tile([128, 2, 4, NSP], f32, name="sfT"),
                "gxi": cst.tile([128, 2, 32], i16, name="gxi"),
                "ghi": cst.tile([128, 4, 32], i16, name="ghi"),
                "idb": cst.tile([128, 128], bf16, name="idb"),
            }
            if has_bias:
                cs["biasT"] = cst.tile([128, 2, NGT], f32, name="biasT")
            for _ in range(reps):
                _build(nc, tc, cs, hid_d, gx_d, gh_d, wihT_d, whhT_d, cmask_d,
                       labs_d, idb_d, biasT_d, out_d, has_bias)
    nc.compile()
    return nc


def _build(nc, tc, cs, hid_d, gx_d, gh_d, wihT_d, whhT_d, cmask_d,
           labs_d, idb_d, biasT_d, out_d, has_bias):
    est = contextlib.ExitStack()
    MM = nc.tensor.matmul

    nc.gpsimd.load_library(library_config.mlp)

    # ---------- constants / persistent ----------
    cmask, sfT = cs["cmask"], cs["sfT"]
    gxi, ghi, idb = cs["gxi"], cs["ghi"], cs["idb"]
    nc.sync.dma_start(cmask[:], cmask_d.ap())
    nc.sync.dma_start(gxi[:], gx_d.ap().rearrange("g p s -> p g s"))
    nc.sync.dma_start(ghi[:], gh_d.ap().rearrange("g p s -> p g s"))
    nc.sync.dma_start(idb[:], idb_d.ap())
    if has_bias:
        biasT = cs["biasT"]
        nc.sync.dma_start(biasT[:], biasT_d.ap())

    nc.sync.dma_start(out_d.ap()[:, 2 * H + D:], labs_d.ap())

    # ---------- weights ----------
    wih_pool = est.enter_context(tc.tile_pool(name="wihp", bufs=1))
    wihT = wih_pool.tile([128, 2, NGT, 8, 128], bf16, name="wihT")
    nc.sync.dma_start(wihT[:], wihT_d.ap())
    wts = est.enter_context(tc.tile_pool(name="wts", bufs=1))
    whhT = wts.tile([128, 2, NGT, 4, 128], bf16, name="whhT")
    nc.sync.dma_start(whhT[:], whhT_d.ap())

    # ---------- gathers: xT (span tokens, transposed) ----------
    hap = hid_d.ap()
    in_ap = bass.AP(tensor=hap.tensor, offset=0, ap=[[D, ROWS], [1, D]])
    xt_pool = est.enter_context(tc.tile_pool(name="xt", bufs=1))
    # xT[p, kc, col]: d = kc*128+p, col = tau*64 + lane (lane = b*16+k)
    xT = xt_pool.tile([128, 8, 1024], bf16, name="xT")
    for q in range(2):
        nc.gpsimd.dma_gather(
            out_ap=xT[:, :, q * 512:(q + 1) * 512], in_ap=in_ap,
            idxs_ap=gxi[:, q, :],
            num_idxs=512, num_idxs_reg=512, elem_size=D, elem_step=D,
            transpose=True, queue_num=q)

    # ---------- phase 1: gxT = WihT @ xT for all steps ----------
    gx_pool = est.enter_context(tc.tile_pool(name="gxp", bufs=1))
    # gxT[p, dir, gt, col]: gate-dim = gt*128+p (perm order g,i,f,o)
    gxT = gx_pool.tile([128, 2, NGT, 1024], bf16, name="gxT")
    with tc.tile_pool(name="p1ps", bufs=4, space="PSUM") as p1ps:
        for d in range(2):
            for gt in range(NGT):
                pg = p1ps.tile([128, 1024], f32, tag="pg", name=f"pg{d}_{gt}")
                for hf in range(2):
                    for kc in range(8):
                        MM(pg[:, hf * 512:(hf + 1) * 512],
                           wihT[:, d, gt, kc, :],
                           xT[:, kc, hf * 512:(hf + 1) * 512],
                           start=(kc == 0), stop=(kc == 7),
                           skip_group_check=True)
                # alternate copy engine to split the load
                if (d * NGT + gt) % 2 == 0:
                    nc.scalar.activation(gxT[:, d, gt, :], pg[:], AF.Copy)
                else:
                    nc.vector.tensor_copy(out=gxT[:, d, gt, :], in_=pg[:])

    # xT and wihT are dead now; hidT gathers + hid_nat loads can land in
    # their space and overlap the recurrence.
    ht_pool = est.enter_context(tc.tile_pool(name="ht", bufs=1))
    hidq = []
    for q in range(4):
        t = ht_pool.tile([128, 8, 512], bf16, name=f"hidq{q}")
        nc.gpsimd.dma_gather(
            out_ap=t[:], in_ap=in_ap, idxs_ap=ghi[:, q, :],
            num_idxs=512, num_idxs_reg=512, elem_size=D, elem_step=D,
            transpose=True, queue_num=q)
        hidq.append(t)

    # ---------- phase 2: LSTM recurrence (transposed gates) ----------
    # pg layout per dir: [128, 16, 64]; gt 0-3 = g (tanh), 4-7 = i,
    # 8-11 = f, 12-15 = o.  c/h: [128, 4, 64] (h-dim = hc*128+p).
    with tc.tile_pool(name="rec", bufs=2) as rec, \
         tc.tile_pool(name="mpsf", bufs=2, space="PSUM") as mpsf, \
         tc.tile_pool(name="mpsb", bufs=2, space="PSUM") as mpsb:
        h_prev = [None, None]
        c_prev = [None, None]
        for tau in range(L):
            for d in range(2):
                pos = tau if d == 0 else (L - 1 - tau)
                col = pos * 64
                sb = {}
                if tau == 0:
                    # gates straight from SBUF; no Whh, no PSUM
                    gsrc = gxT[:, d, :, col:col + 64]
                else:
                    mp = mpsf if d == 0 else mpsb
                    pg = mp.tile([128, NGT, 64], f32, tag=f"pg{d}",
                                 name=f"pg{d}_{tau}")
                    # inject gxT (start=True), interleaved with Whh
                    # per 4-gt quarter so g's quarter finishes early
                    for q in range(4):
                        MM(pg[:, 4 * q:4 * q + 4, :], idb[:],
                           gxT[:, d, 4 * q:4 * q + 4, col:col + 64],
                           start=True, stop=False, skip_group_check=True)
                        for gt in range(4 * q, 4 * q + 4):
                            for kc in range(4):
                                MM(pg[:, gt, :], whhT[:, d, gt, kc, :],
                                   h_prev[d][:, kc, :],
                                   start=False, stop=(kc == 3),
                                   skip_group_check=True)
                    gsrc = pg[:, :, :]
                bias_kw = {}
                tg = rec.tile([128, 4, 64], bf16, tag=f"tg{d}")
                if has_bias:
                    nc.scalar.activation(tg[:], gsrc[:, 0:4, :], AF.Tanh,
                                         bias=cs["biasT"][:, d, 0:1])
                else:
                    nc.scalar.activation(tg[:], gsrc[:, 0:4, :], AF.Tanh)
                sg = rec.tile([128, 12, 64], bf16, tag=f"sg{d}")
                for part in range(3):
                    gslc = gsrc[:, 4 * (part + 1):4 * (part + 2), :]
                    if has_bias:
                        nc.scalar.activation(
                            sg[:, 4 * part:4 * part + 4, :], gslc,
                            AF.Sigmoid, bias=cs["biasT"][:, d, part + 1:part + 2])
                    else:
                        nc.scalar.activation(
                            sg[:, 4 * part:4 * part + 4, :], gslc, AF.Sigmoid)
                ig = rec.tile([128, 4, 64], bf16, tag=f"ig{d}")
                nc.vector.tensor_tensor(out=ig[:], in0=sg[:, 0:4, :],
                                        in1=tg[:], op=OP.mult)
                c_new = rec.tile([128, 4, 64], f32, tag=f"c{d}")
                if tau == 0:
                    nc.vector.tensor_copy(out=c_new[:], in_=ig[:])
                else:
                    fc = rec.tile([128, 4, 64], f32, tag=f"fc{d}")
                    nc.vector.tensor_tensor(out=fc[:], in0=sg[:, 4:8, :],
                                            in1=c_prev[d][:], op=OP.mult)
                    nc.vector.tensor_tensor(out=c_new[:], in0=ig[:],
                                            in1=fc[:], op=OP.add)
                th = rec.tile([128, 4, 64], bf16, tag=f"th{d}")
                nc.scalar.activation(th[:], c_new[:], AF.Tanh)
                h_new = rec.tile([128, 4, 64], bf16, tag=f"h{d}")
                nc.vector.tensor_tensor(out=h_new[:], in0=sg[:, 8:12, :],
                                        in1=th[:], op=OP.mult)
                if tau == 0:
                    nc.vector.tensor_copy(out=sfT[:, d, :, :], in_=h_new[:])
                else:
                    nc.vector.tensor_tensor(out=sfT[:, d, :, :],
                                            in0=sfT[:, d, :, :],
                                            in1=h_new[:], op=OP.add)
                h_prev[d] = h_new
                c_prev[d] = c_new

    # ---------- slot_feats out (transpose sfT -> [64, 1024]) ----------
    with tc.tile_pool(name="sfo", bufs=1) as sfo:
        sf_bf = sfo.tile([128, 2, 4, NSP], bf16)
        nc.vector.tensor_copy(out=sf_bf[:], in_=sfT[:])
        sfn = sfo.tile([NSP, 8, 128], bf16)
        for j in range(8):
            d, hc = divmod(j, 4)
            nc.sync.dma_start_transpose(sfn[:, j, :], sf_bf[:, d, hc, :])
        sfn32 = sfo.tile([NSP, 2 * H], f32)
        nc.scalar.activation(sfn32[:], sfn[:], AF.Copy)
        nc.sync.dma_start(out_d.ap()[:, 0:2 * H], sfn32[:])

        # ---------- attention (baseline structure) ----------
        with tc.tile_pool(name="asb", bufs=1) as asb, \
             tc.tile_pool(name="aps", bufs=1, space="PSUM") as aps:
            # pipelined by sample-quarter: attn(nq) -> mask -> transpose ->
            # ctx MMs for that quarter run while quarter nq+1 accumulates
            ps_at = aps.tile([NSP, BL * S], f32)
            at2 = asb.tile([NSP, BL * S], bf16)
            atT = asb.tile([128, 16, 64], bf16)
            ps_ctx = aps.tile([NSP, D], f32)
            ctx_sb = asb.tile([NSP, D], f32)
            with tc.tile_pool(name="hnat", bufs=8) as hnat:
                for nq in range(BL):
                    qs = slice(nq * 512, (nq + 1) * 512)
                    for dc in range(8):
                        d, hc = divmod(dc, 4)
                        MM(ps_at[:, qs], sf_bf[:, d, hc, :],
                           hidq[nq][:, dc, :],
                           start=(dc == 0), stop=(dc == 7),
                           skip_group_check=True)
                    nc.vector.tensor_tensor(out=at2[:, qs], in0=ps_at[:, qs],
                                            in1=cmask[:, qs], op=OP.mult)
                    nc.sync.dma_start_transpose(
                        atT[:, 4 * nq: 4 * nq + 4, :], at2[:, qs])
                    for si in range(4):
                        sc = 4 * nq + si
                        hn = hnat.tile([128, D], bf16, tag="hn", name=f"hn{sc}")
                        nc.sync.dma_start(hn[:], hap[sc * 128:(sc + 1) * 128, :])
                        for f2 in range(2):
                            MM(ps_ctx[:, f2 * 512:(f2 + 1) * 512],
                               atT[:, sc, :],
                               hn[:, f2 * 512:(f2 + 1) * 512],
                               start=(sc == 0), stop=(sc == 15),
                               skip_group_check=True)
            nc.vector.tensor_copy(out=ctx_sb[:], in_=ps_ctx[:])
            nc.sync.dma_start(out_d.ap()[:, 2 * H: 2 * H + D], ctx_sb[:])

    est.close()


# ---------------- host side ----------------

def _mlp_np(x, W1, b1, W2, b2):
    return np.tanh(x @ W1.T + b1) @ W2.T + b2


def _wrap_idx(idx512):
    g = np.zeros((16, 32), np.int16)
    for i in range(512):
        g[i % 16, i // 16] = idx512[i]
    return np.tile(g, (8, 1))


def prep_core_inputs(inp, ci):
    b0 = ci * BL
    hid = np.asarray(inp["hidden_layers"][b0:b0 + BL],
                     np.float32).reshape(ROWS, D)
    hid_bf = np.ascontiguousarray(hid.astype(ml_dtypes.bfloat16))

    span_idx = np.asarray(inp["span_idx"][b0:b0 + BL], np.int64)  # [BL,K,L]
    # xT token order: col = tau*64 + lane, lane = b*16 + k
    gx = np.zeros((2, 128, 32), np.int16)
    for q in range(2):
        idxs = np.zeros(512, np.int64)
        for i in range(512):
            gcol = q * 512 + i
            tau, lane = divmod(gcol, 64)
            b, k = divmod(lane, K)
            idxs[i] = b * S + span_idx[b, k, tau]
        gx[q] = _wrap_idx(idxs)
    gh = np.zeros((4, 128, 32), np.int16)
    for tq in range(4):
        gh[tq] = _wrap_idx(np.arange(tq * 512, (tq + 1) * 512))

    def wT(w, nkc):  # [4H, Din] -> [128, NGT, nkc, 128] transposed chunks
        wt = np.asarray(w, np.float32)
        din = wt.shape[1]
        wp = wt.reshape(4, H, din)[list(PERM)].reshape(G4, din)  # [2048, din]
        # out[p, gt, kc, m] = wp[gt*128+m, kc*128+p]
        o = wp.reshape(NGT, 128, nkc, 128).transpose(3, 0, 2, 1)
        return np.ascontiguousarray(o)

    wihT = np.stack([wT(inp["Wih_f"], 8), wT(inp["Wih_b"], 8)], axis=1)
    whhT = np.stack([wT(inp["Whh_f"], 4), wT(inp["Whh_b"], 4)], axis=1)

    def bperm(bih, bhh):
        v = (np.asarray(bih, np.float32) + np.asarray(bhh, np.float32))
        return v.reshape(4, H)[list(PERM)].reshape(G4)

    bias2 = np.stack([bperm(inp["bih_f"], inp["bhh_f"]),
                      bperm(inp["bih_b"], inp["bhh_b"])])  # [2, 2048]
    has_bias = bool(np.any(bias2 != 0.0))
    # biasT[p, dir, gate-type] (bias is constant over the 4 h-chunks of a
    # gate type only if ... it is NOT; transposed bias needs [p, dir, gt])
    biasT = np.ascontiguousarray(
        bias2.reshape(2, NGT, 128).transpose(2, 0, 1).astype(np.float32))

    # context mask, block-diagonal over samples: [64, BL, S]
    ss = np.asarray(inp["span_start"][b0:b0 + BL], np.int64)
    se = np.asarray(inp["span_end"][b0:b0 + BL], np.int64)
    ln = np.asarray(inp["length"][b0:b0 + BL], np.int64)
    pos = np.arange(S)
    cmask = np.zeros((BL, K, BL, S), np.float32)
    for b in range(BL):
        m = ((pos[None, :] < ss[b][:, None])
             | ((pos[None, :] > se[b][:, None])
                & (pos[None, :] < ln[b])))
        cmask[b, :, b, :] = m
    cmask = cmask.reshape(NSP, BL, S)

    # labels on host (fp32): one-hot*SMOOTH | sim normalized
    se_ = np.asarray(inp["slot_emb"][b0:b0 + BL], np.float32).reshape(NSP, D)
    tgt = np.asarray(inp["tgt_slot_embs"], np.float32)

    def mlp32(x, w1, bb1, w2, bb2):
        return _mlp_np(x, np.asarray(w1, np.float32),
                       np.asarray(bb1, np.float32),
                       np.asarray(w2, np.float32),
                       np.asarray(bb2, np.float32))

    s_cat = np.concatenate([
        mlp32(se_, inp["Wps1"], inp["bps1"], inp["Wps2"], inp["bps2"]),
        mlp32(se_, inp["Wpc1"], inp["bpc1"], inp["Wpc2"], inp["bpc2"])],
        axis=-1)
    t_cat = np.concatenate([
        mlp32(tgt, inp["Wps1"], inp["bps1"], inp["Wps2"], inp["bps2"]),
        mlp32(tgt, inp["Wpc1"], inp["bpc1"], inp["Wpc2"], inp["bpc2"])],
        axis=-1)
    sn = np.maximum(np.linalg.norm(s_cat, axis=-1), EPS)
    tn = np.maximum(np.linalg.norm(t_cat, axis=-1), EPS)
    sim = (s_cat @ t_cat.T) / (sn[:, None] * tn[None, :])
    labsim = (sim / sim.sum(axis=-1, keepdims=True) * (1.0 - SMOOTH))
    sid = np.asarray(inp["src_slot_ids"][b0:b0 + BL], np.int64).reshape(NSP)
    oh = np.zeros((NSP, NS), np.float32)
    oh[np.arange(NSP), sid] = SMOOTH
    labs = np.concatenate([oh, labsim.astype(np.float32)], axis=1)

    def bf(a):
        return np.ascontiguousarray(np.asarray(a).astype(ml_dtypes.bfloat16))

    return {
        "hid": hid_bf, "gx": gx, "gh": gh,
        "wihT": bf(wihT), "whhT": bf(whhT),
        "cmask": bf(cmask), "labs": labs.astype(np.float32),
        "idb": bf(np.eye(128)), "biasT": biasT,
    }, has_bias


_NC_CACHE = {}


def _get_nc(has_bias=False):
    if has_bias not in _NC_CACHE:
        _NC_CACHE[has_bias] = build_program(has_bias=has_bias)
    return _NC_CACHE[has_bias]


def kernel(**inputs):
    preps = [prep_core_inputs(inputs, ci) for ci in range(NCORES)]
    has_bias = any(p[1] for p in preps)
    in_maps = [p[0] for p in preps]
    nc = _get_nc(has_bias)
    res = bass_utils.run_bass_kernel_spmd(nc, in_maps, list(range(NCORES)))
    outs = [res.results[i]["out"].reshape(BL, K, OUTW) for i in range(NCORES)]
    return np.concatenate(outs, axis=0)


# revision 6
# speedup vs baseline: 1.0355x; 1.0355x over previous
"""Trainium2 Bass kernel for nn_BertContrastivePredictor.

Sharding: data-parallel over batch, 4 samples per core (8 cores).
Per core: 64 spans (fwd + bwd chains).

Structure (v2 — hoisted Wih, transposed gates):
  - Phase 1: gxT[dir, gt, :] = WihT-chunk @ xT for ALL 16 steps at once.
    Full 128x128 stationary (weight chunks), moving = xT token columns
    (N=512) -> stream-bound at the bf16 roofline (~109us model).
  - Phase 2: recurrence in gates-TRANSPOSED layout [gate-dim, span].
    Per step per dir: inject gxT slice via identity-stationary matmuls
    (start=True), then 64 Whh matmuls [128,128]x[128,64] accumulate.
    h stays transposed -> no per-step DMA transposes; c/h/sf updates on
    scalar+vector engines overlap the other direction's PE work.
  - Phase 3: attention identical to baseline (block-diagonal masked
    attn, pipelined by sample), but sfT now exists natively.

labels sim-part on host (precision; ~0.5% of FLOPs), as baseline.

Output per core [64, 2080] = [slot_feats(1024) | context(1024) | labels(32)].
"""

import contextlib

import numpy as np
import ml_dtypes

import concourse.bass as bass
import concourse.bacc as bacc
import concourse.tile as tile
import concourse.mybir as mybir
from concourse import bass_utils
from concourse import library_config

f32 = mybir.dt.float32
bf16 = mybir.dt.bfloat16
i16 = mybir.dt.int16
AF = mybir.ActivationFunctionType
OP = mybir.AluOpType

B, S, D, H, K, L, NS, NT = 32, 512, 1024, 512, 16, 16, 16, 16
SMOOTH = 0.1
EPS = 1e-8
NCORES = 8
BL = B // NCORES            # local batch = 4
NSP = BL * K                # local spans = 64
G4 = 4 * H                  # 2048 gates per direction
OUTW = 2 * H + D + NS + NT  # 2080
ROWS = BL * S               # 2048 hidden rows per core
PERM = (2, 0, 1, 3)         # torch gate order (i,f,g,o) -> (g,i,f,o)
NGT = 16                    # gate tiles of 128 per direction


def build_program(dbg=False, reps=1, has_bias=False):
    nc = bacc.Bacc("TRN2", target_bir_lowering=False, debug=False,
                   num_swdge_queues=4)

    hid_d = nc.dram_tensor("hid", [ROWS, D], bf16, kind="ExternalInput")
    gx_d = nc.dram_tensor("gx", [2, 128, 32], i16, kind="ExternalInput")
    gh_d = nc.dram_tensor("gh", [4, 128, 32], i16, kind="ExternalInput")
    wihT_d = nc.dram_tensor("wihT", [128, 2, NGT, 8, 128], bf16,
                            kind="ExternalInput")
    whhT_d = nc.dram_tensor("whhT", [128, 2, NGT, 4, 128], bf16,
                            kind="ExternalInput")
    cmask_d = nc.dram_tensor("cmask", [NSP, BL, S], bf16, kind="ExternalInput")
    labs_d = nc.dram_tensor("labs", [NSP, NS + NT], f32, kind="ExternalInput")
    idb_d = nc.dram_tensor("idb", [128, 128], bf16, kind="ExternalInput")
    biasT_d = nc.dram_tensor("biasT", [128, 2, NGT], f32, kind="ExternalInput")
    out_d = nc.dram_tensor("out", [NSP, OUTW], f32, kind="ExternalOutput")

    with tile.TileContext(nc, pool_alloc_mode="queue") as tc:
        with tc.tile_pool(name="cst", bufs=1) as cst:
            cs = {
                "cmask": cst.tile([NSP, BL * S], bf16, name="cmask"),
                "sfT": cst.tile([128, 2, 4, NSP], f32, name="sfT"),
                "gxi": cst.tile([128, 2, 32], i16, name="gxi"),
                "ghi": cst.tile([128, 4, 32], i16, name="ghi"),
                "idb": cst.tile([128, 128], bf16, name="idb"),
            }
            if has_bias:
                cs["biasT"] = cst.tile([128, 2, NGT], f32, name="biasT")
            for _ in range(reps):
                _build(nc, tc, cs, hid_d, gx_d, gh_d, wihT_d, whhT_d, cmask_d,
                       labs_d, idb_d, biasT_d, out_d, has_bias)
    nc.compile()
    return nc


def _build(nc, tc, cs, hid_d, gx_d, gh_d, wihT_d, whhT_d, cmask_d,
           labs_d, idb_d, biasT_d, out_d, has_bias):
    est = contextlib.ExitStack()
    MM = nc.tensor.matmul

    nc.gpsimd.load_library(library_config.mlp)

    # ---------- constants / persistent ----------
    cmask, sfT = cs["cmask"], cs["sfT"]
    gxi, ghi, idb = cs["gxi"], cs["ghi"], cs["idb"]
    nc.sync.dma_start(cmask[:], cmask_d.ap())
    nc.sync.dma_start(gxi[:], gx_d.ap().rearrange("g p s -> p g s"))
    nc.sync.dma_start(ghi[:], gh_d.ap().rearrange("g p s -> p g s"))
    nc.sync.dma_start(idb[:], idb_d.ap())
    if has_bias:
        biasT = cs["biasT"]
        nc.sync.dma_start(biasT[:], biasT_d.ap())

    nc.sync.dma_start(out_d.ap()[:, 2 * H + D:], labs_d.ap())

    # ---------- weights ----------
    wts = est.enter_context(tc.tile_pool(name="wts", bufs=1))
    whhT = wts.tile([128, 2, NGT, 4, 128], bf16, name="whhT")
    nc.sync.dma_start(whhT[:], whhT_d.ap())

    hap = hid_d.ap()
    in_ap = bass.AP(tensor=hap.tensor, offset=0, ap=[[D, ROWS], [1, D]])

    # ---------- phase 1: gxT = WihT @ xT for all steps ----------
    gx_pool = est.enter_context(tc.tile_pool(name="gxp", bufs=1))
    # gxT[p, dir, gt, col]: gate-dim = gt*128+p (perm order g,i,f,o)
    gxT = gx_pool.tile([128, 2, NGT, 1024], bf16, name="gxT")
    # wihT and xT live only for phase 1; closing their pools frees the
    # space for the hidT gathers + attention tiles.
    with tc.tile_pool(name="wihp", bufs=1) as wih_pool, \
         tc.tile_pool(name="xt", bufs=1) as xt_pool, \
         tc.tile_pool(name="p1ps", bufs=4, space="PSUM") as p1ps:
        wihT = wih_pool.tile([128, 2, NGT, 8, 128], bf16, name="wihT")
        nc.sync.dma_start(wihT[:], wihT_d.ap())
        # xT[p, kc, col]: d = kc*128+p, col = tau*64 + lane (lane = b*16+k)
        xtq = []
        for q in range(2):
            t = xt_pool.tile([128, 8, 512], bf16, name=f"xtq{q}")
            nc.gpsimd.dma_gather(
                out_ap=t[:], in_ap=in_ap, idxs_ap=gxi[:, q, :],
                num_idxs=512, num_idxs_reg=512, elem_size=D, elem_step=D,
                transpose=True, queue_num=q)
            xtq.append(t)
        for d in range(2):
            for gt in range(NGT):
                pg = p1ps.tile([128, 1024], f32, tag="pg", name=f"pg{d}_{gt}")
                for hf in range(2):
                    for kc in range(8):
                        MM(pg[:, hf * 512:(hf + 1) * 512],
                           wihT[:, d, gt, kc, :],
                           xtq[hf][:, kc, :],
                           start=(kc == 0), stop=(kc == 7),
                           skip_group_check=True)
                # alternate copy engine to split the load
                if (d * NGT + gt) % 2 == 0:
                    nc.scalar.activation(gxT[:, d, gt, :], pg[:], AF.Copy)
                else:
                    nc.vector.tensor_copy(out=gxT[:, d, gt, :], in_=pg[:])

    # xT and wihT are dead now; hidT gathers + hid_nat loads can land in
    # their space and overlap the recurrence.
    ht_pool = est.enter_context(tc.tile_pool(name="ht", bufs=1))
    hidq = []
    for q in range(4):
        t = ht_pool.tile([128, 8, 512], bf16, name=f"hidq{q}")
        nc.gpsimd.dma_gather(
            out_ap=t[:], in_ap=in_ap, idxs_ap=ghi[:, q, :],
            num_idxs=512, num_idxs_reg=512, elem_size=D, elem_step=D,
            transpose=True, queue_num=q)
        hidq.append(t)

    # ---------- phase 2: LSTM recurrence (transposed gates) ----------
    # pg layout per dir: [128, 16, 64]; gt 0-3 = g (tanh), 4-7 = i,
    # 8-11 = f, 12-15 = o.  c/h: [128, 4, 64] (h-dim = hc*128+p).
    with tc.tile_pool(name="rec", bufs=2) as rec, \
         tc.tile_pool(name="mpsf", bufs=2, space="PSUM") as mpsf, \
         tc.tile_pool(name="mpsb", bufs=2, space="PSUM") as mpsb:
        h_prev = [None, None]
        c_prev = [None, None]
        for tau in range(L):
            for d in range(2):
                pos = tau if d == 0 else (L - 1 - tau)
                col = pos * 64
                sb = {}
                if tau == 0:
                    # gates straight from SBUF; no Whh, no PSUM
                    gsrc = gxT[:, d, :, col:col + 64]
                else:
                    mp = mpsf if d == 0 else mpsb
                    pg = mp.tile([128, NGT, 64], f32, tag=f"pg{d}",
                                 name=f"pg{d}_{tau}")
                    # inject gxT (start=True), interleaved with Whh
                    # per 4-gt quarter so g's quarter finishes early
                    for q in range(4):
                        MM(pg[:, 4 * q:4 * q + 4, :], idb[:],
                           gxT[:, d, 4 * q:4 * q + 4, col:col + 64],
                           start=True, stop=False, skip_group_check=True)
                        for gt in range(4 * q, 4 * q + 4):
                            for kc in range(4):
                                MM(pg[:, gt, :], whhT[:, d, gt, kc, :],
                                   h_prev[d][:, kc, :],
                                   start=False, stop=(kc == 3),
                                   skip_group_check=True)
                    gsrc = pg[:, :, :]
                tg = rec.tile([128, 4, 64], bf16, tag=f"tg{d}")
                sg = rec.tile([128, 12, 64], bf16, tag=f"sg{d}")
                if has_bias:
                    # bias differs per gate-dim -> one act per 128-gt tile
                    for gt in range(4):
                        nc.scalar.activation(
                            tg[:, gt, :], gsrc[:, gt, :], AF.Tanh,
                            bias=cs["biasT"][:, d, gt:gt + 1])
                    for gt in range(4, NGT):
                        nc.scalar.activation(
                            sg[:, gt - 4, :], gsrc[:, gt, :], AF.Sigmoid,
                            bias=cs["biasT"][:, d, gt:gt + 1])
                else:
                    nc.scalar.activation(tg[:], gsrc[:, 0:4, :], AF.Tanh)
                    for part in range(3):
                        nc.scalar.activation(
                            sg[:, 4 * part:4 * part + 4, :],
                            gsrc[:, 4 * (part + 1):4 * (part + 2), :],
                            AF.Sigmoid)
                ig = rec.tile([128, 4, 64], bf16, tag=f"ig{d}")
                nc.vector.tensor_tensor(out=ig[:], in0=sg[:, 0:4, :],
                                        in1=tg[:], op=OP.mult)
                c_new = rec.tile([128, 4, 64], f32, tag=f"c{d}")
                if tau == 0:
                    nc.vector.tensor_copy(out=c_new[:], in_=ig[:])
                else:
                    fc = rec.tile([128, 4, 64], f32, tag=f"fc{d}")
                    nc.vector.tensor_tensor(out=fc[:], in0=sg[:, 4:8, :],
                                            in1=c_prev[d][:], op=OP.mult)
                    nc.vector.tensor_tensor(out=c_new[:], in0=ig[:],
                                            in1=fc[:], op=OP.add)
                th = rec.tile([128, 4, 64], bf16, tag=f"th{d}")
                nc.scalar.activation(th[:], c_new[:], AF.Tanh)
                h_new = rec.tile([128, 4, 64], bf16, tag=f"h{d}")
                nc.vector.tensor_tensor(out=h_new[:], in0=sg[:, 8:12, :],
                                        in1=th[:], op=OP.mult)
                if tau == 0:
                    nc.vector.tensor_copy(out=sfT[:, d, :, :], in_=h_new[:])
                else:
                    nc.vector.tensor_tensor(out=sfT[:, d, :, :],
                                            in0=sfT[:, d, :, :],
                                            in1=h_new[:], op=OP.add)
                h_prev[d] = h_new
                c_prev[d] = c_new

    # ---------- slot_feats out (transpose sfT -> [64, 1024]) ----------
    with tc.tile_pool(name="sfo", bufs=1) as sfo:
        sf_bf = sfo.tile([128, 2, 4, NSP], bf16)
        nc.vector.tensor_copy(out=sf_bf[:], in_=sfT[:])
        # xbar transpose in 128x128 blocks: block b = (d, hc-pair); rows
        # 0-63 = spans of even hc, 64-127 = spans of odd hc
        sfn = sfo.tile([128, 4, 128], bf16)
        nc.sync.dma_start_transpose(sfn[:], sf_bf[:])
        sfn32 = sfo.tile([128, 4, 128], f32)
        nc.scalar.activation(sfn32[:], sfn[:], AF.Copy)
        for blk in range(4):
            d, hp = divmod(blk, 2)
            for half in range(2):
                base = d * 512 + (2 * hp + half) * 128
                nc.sync.dma_start(
                    out_d.ap()[:, base:base + 128],
                    sfn32[64 * half:64 * half + 64, blk, :])

        # ---------- attention (baseline structure) ----------
        with tc.tile_pool(name="asb", bufs=1) as asb, \
             tc.tile_pool(name="aps", bufs=1, space="PSUM") as aps:
            # pipelined by sample-quarter: attn(nq) -> mask -> transpose ->
            # ctx MMs for that quarter run while quarter nq+1 accumulates
            ps_at = aps.tile([NSP, BL * S], f32)
            at2 = asb.tile([NSP, BL * S], bf16)
            atT = asb.tile([128, 16, 64], bf16)
            ps_ctx = aps.tile([NSP, D], f32)
            ctx_sb = asb.tile([NSP, D], f32)
            with tc.tile_pool(name="hnat", bufs=8) as hnat:
                for nq in range(BL):
                    qs = slice(nq * 512, (nq + 1) * 512)
                    for dc in range(8):
                        d, hc = divmod(dc, 4)
                        MM(ps_at[:, qs], sf_bf[:, d, hc, :],
                           hidq[nq][:, dc, :],
                           start=(dc == 0), stop=(dc == 7),
                           skip_group_check=True)
                    nc.vector.tensor_tensor(out=at2[:, qs], in0=ps_at[:, qs],
                                            in1=cmask[:, qs], op=OP.mult)
                    nc.sync.dma_start_transpose(
                        atT[:, 4 * nq: 4 * nq + 4, :], at2[:, qs])
                    for si in range(4):
                        sc = 4 * nq + si
                        hn = hnat.tile([128, D], bf16, tag="hn", name=f"hn{sc}")
                        nc.sync.dma_start(hn[:], hap[sc * 128:(sc + 1) * 128, :])
                        for f2 in range(2):
                            MM(ps_ctx[:, f2 * 512:(f2 + 1) * 512],
                               atT[:, sc, :],
                               hn[:, f2 * 512:(f2 + 1) * 512],
                               start=(sc == 0), stop=(sc == 15),
                               skip_group_check=True)
            nc.vector.tensor_copy(out=ctx_sb[:], in_=ps_ctx[:])
            nc.sync.dma_start(out_d.ap()[:, 2 * H: 2 * H + D], ctx_sb[:])

    est.close()


# ---------------- host side ----------------

def _mlp_np(x, W1, b1, W2, b2):
    return np.tanh(x @ W1.T + b1) @ W2.T + b2


def _wrap_idx(idx512):
    g = np.zeros((16, 32), np.int16)
    for i in range(512):
        g[i % 16, i // 16] = idx512[i]
    return np.tile(g, (8, 1))


def prep_core_inputs(inp, ci):
    b0 = ci * BL
    hid = np.asarray(inp["hidden_layers"][b0:b0 + BL],
                     np.float32).reshape(ROWS, D)
    hid_bf = np.ascontiguousarray(hid.astype(ml_dtypes.bfloat16))

    span_idx = np.asarray(inp["span_idx"][b0:b0 + BL], np.int64)  # [BL,K,L]
    # xT token order: col = tau*64 + lane, lane = b*16 + k
    gx = np.zeros((2, 128, 32), np.int16)
    for q in range(2):
        idxs = np.zeros(512, np.int64)
        for i in range(512):
            gcol = q * 512 + i
            tau, lane = divmod(gcol, 64)
            b, k = divmod(lane, K)
            idxs[i] = b * S + span_idx[b, k, tau]
        gx[q] = _wrap_idx(idxs)
    gh = np.zeros((4, 128, 32), np.int16)
    for tq in range(4):
        gh[tq] = _wrap_idx(np.arange(tq * 512, (tq + 1) * 512))

    def wT(w, nkc):  # [4H, Din] -> [128, NGT, nkc, 128] transposed chunks
        wt = np.asarray(w, np.float32)
        din = wt.shape[1]
        wp = wt.reshape(4, H, din)[list(PERM)].reshape(G4, din)  # [2048, din]
        # out[p, gt, kc, m] = wp[gt*128+m, kc*128+p]
        o = wp.reshape(NGT, 128, nkc, 128).transpose(3, 0, 2, 1)
        return np.ascontiguousarray(o)

    wihT = np.stack([wT(inp["Wih_f"], 8), wT(inp["Wih_b"], 8)], axis=1)
    whhT = np.stack([wT(inp["Whh_f"], 4), wT(inp["Whh_b"], 4)], axis=1)

    def bperm(bih, bhh):
        v = (np.asarray(bih, np.float32) + np.asarray(bhh, np.float32))
        return v.reshape(4, H)[list(PERM)].reshape(G4)

    bias2 = np.stack([bperm(inp["bih_f"], inp["bhh_f"]),
                      bperm(inp["bih_b"], inp["bhh_b"])])  # [2, 2048]
    has_bias = bool(np.any(bias2 != 0.0))
    # biasT[p, dir, gate-type] (bias is constant over the 4 h-chunks of a
    # gate type only if ... it is NOT; transposed bias needs [p, dir, gt])
    biasT = np.ascontiguousarray(
        bias2.reshape(2, NGT, 128).transpose(2, 0, 1).astype(np.float32))

    # context mask, block-diagonal over samples: [64, BL, S]
    ss = np.asarray(inp["span_start"][b0:b0 + BL], np.int64)
    se = np.asarray(inp["span_end"][b0:b0 + BL], np.int64)
    ln = np.asarray(inp["length"][b0:b0 + BL], np.int64)
    pos = np.arange(S)
    cmask = np.zeros((BL, K, BL, S), np.float32)
    for b in range(BL):
        m = ((pos[None, :] < ss[b][:, None])
             | ((pos[None, :] > se[b][:, None])
                & (pos[None, :] < ln[b])))
        cmask[b, :, b, :] = m
    cmask = cmask.reshape(NSP, BL, S)

    # labels on host (fp32): one-hot*SMOOTH | sim normalized
    se_ = np.asarray(inp["slot_emb"][b0:b0 + BL], np.float32).reshape(NSP, D)
    tgt = np.asarray(inp["tgt_slot_embs"], np.float32)

    def mlp32(x, w1, bb1, w2, bb2):
        return _mlp_np(x, np.asarray(w1, np.float32),
                       np.asarray(bb1, np.float32),
                       np.asarray(w2, np.float32),
                       np.asarray(bb2, np.float32))

    s_cat = np.concatenate([
        mlp32(se_, inp["Wps1"], inp["bps1"], inp["Wps2"], inp["bps2"]),
        mlp32(se_, inp["Wpc1"], inp["bpc1"], inp["Wpc2"], inp["bpc2"])],
        axis=-1)
    t_cat = np.concatenate([
        mlp32(tgt, inp["Wps1"], inp["bps1"], inp["Wps2"], inp["bps2"]),
        mlp32(tgt, inp["Wpc1"], inp["bpc1"], inp["Wpc2"], inp["bpc2"])],
        axis=-1)
    sn = np.maximum(np.linalg.norm(s_cat, axis=-1), EPS)
    tn = np.maximum(np.linalg.norm(t_cat, axis=-1), EPS)
    sim = (s_cat @ t_cat.T) / (sn[:, None] * tn[None, :])
    labsim = (sim / sim.sum(axis=-1, keepdims=True) * (1.0 - SMOOTH))
    sid = np.asarray(inp["src_slot_ids"][b0:b0 + BL], np.int64).reshape(NSP)
    oh = np.zeros((NSP, NS), np.float32)
    oh[np.arange(NSP), sid] = SMOOTH
    labs = np.concatenate([oh, labsim.astype(np.float32)], axis=1)

    def bf(a):
        return np.ascontiguousarray(np.asarray(a).astype(ml_dtypes.bfloat16))

    return {
        "hid": hid_bf, "gx": gx, "gh": gh,
        "wihT": bf(wihT), "whhT": bf(whhT),
        "cmask": bf(cmask), "labs": labs.astype(np.float32),
        "idb": bf(np.eye(128)), "biasT": biasT,
    }, has_bias


_NC_CACHE = {}


def _get_nc(has_bias=False):
    if has_bias not in _NC_CACHE:
        _NC_CACHE[has_bias] = build_program(has_bias=has_bias)
    return _NC_CACHE[has_bias]


def kernel(**inputs):
    preps = [prep_core_inputs(inputs, ci) for ci in range(NCORES)]
    has_bias = any(p[1] for p in preps)
    in_maps = [p[0] for p in preps]
    nc = _get_nc(has_bias)
    res = bass_utils.run_bass_kernel_spmd(nc, in_maps, list(range(NCORES)))
    outs = [res.results[i]["out"].reshape(BL, K, OUTW) for i in range(NCORES)]
    return np.concatenate(outs, axis=0)
